# revision 4
# baseline (speedup 1.0000x reference)
"""Trainium2 Bass kernel for nn_Attention_48825188221088.

  out     = lstm_out @ W.T + b        [B,S,H]
  score   = out @ out.T (per batch)   [B,S,S]
  attn    = softmax(score, -1)
  context = attn @ lstm_out           [B,S,H]

B=8, S=2048, H=1024, fp32 I/O. Sharding: data-parallel over batch B across
the 8 NeuronCores (one batch element per core); no collectives.

Per-core kernel — every matmul is fp8e4 DoubleRow on the PE (fp32 PSUM):
  1. W*32 -> Wt (fp8, [h,o]) via PE transposes; b -> per-partition bias.
  2. x -> x_h8 = fp8(x) and x_l8 = fp8(256*(x - x_h8)) (2-term residual
     split, ~6e-4 quantization error); x_h8 -> xT via PE transposes.
  3. Linear: outT[o,s] = (Wt.T @ xT)/32 + b, fp8 DoubleRow.
  4. Per 128-row q-block: score (fp8 DR) -> row-max (DVE) -> exp straight
     from PSUM (ACT, accum_out row-sum) -> attnT via DMA-xbar transpose ->
     fp8 cast -> context = attnT.T @ x_h8 + (attnT.T @ x_l8)/256, both fp8
     DR, scaled by 1/rowsum at PSUM eviction -> DMA out.
"""

import os
from contextlib import ExitStack

import numpy as np

import concourse.bass as bass
import concourse.mybir as mybir
import concourse.tile as tile
from concourse import bacc
from concourse.bass_utils import run_bass_kernel_spmd
from concourse.masks import make_identity

B, S, H = 8, 2048, 1024
P = 128  # SBUF/PSUM partitions
F = 512  # matmul free dim = one PSUM bank of fp32
SQ = S // P  # 16 s-blocks of 128
HC = H // P  # 8 h-blocks of 128
NK = S // F  # 4 score column chunks of 512
NH = H // F  # 2 context h chunks of 512

f32 = mybir.dt.float32
bf16 = mybir.dt.bfloat16
f8 = mybir.dt.float8e4

def _flag(name, default):
    v = os.environ.get("ATTN_" + name)
    return default if v is None else eval(v)


W_SCALE = 32.0
PCTX_BUFS = _flag("PCTX_BUFS", 2)
EXP_PSUM = _flag("EXP_PSUM", True)  # ACT exp reads score PSUM directly
XTR_F8 = _flag("XTR_F8", True)  # transpose x_h8 (fp8) directly on the PE
SUB_ENGINE = _flag("SUB_ENGINE", "pool")  # residual sub engine
AT8_ENGINE = _flag("AT8_ENGINE", "pool")  # attnT bf16->fp8 cast engine


def emit_iteration(nc, tc, x, W, b, out, psum, const, ident, ident_f8, b_sb, it=0):
    """Emit one full attention pass over a single batch element."""
    sub_eng = {"pool": nc.gpsimd, "vector": nc.vector}[SUB_ENGINE]
    at8_eng = {"pool": nc.gpsimd, "vector": nc.vector}[AT8_ENGINE]
    with ExitStack() as top:
        persist = top.enter_context(tc.tile_pool(name=f"persist{it}", bufs=1))

        # --- Phase W+X interleaved: Wt, x_h8/x_l8, xT ---------------------
        Wt = persist.tile([P, HC, H], f8, name=f"Wt{it}")
        x_h8 = persist.tile([P, SQ, H], f8, name=f"x_h8{it}")
        x_l8 = persist.tile([P, SQ, H], f8, name=f"x_l8{it}")
        outT = persist.tile([P, HC, S], f8, name=f"outT{it}")

        with ExitStack() as linscope:
            xtp = linscope.enter_context(tc.tile_pool(name=f"xtp{it}", bufs=1))
            xT = xtp.tile([P, HC, S], f8, name=f"xT{it}")
            stage = linscope.enter_context(tc.tile_pool(name=f"stage{it}", bufs=6))
            wstage = linscope.enter_context(
                tc.tile_pool(name=f"wstage{it}", bufs=3)
            )

            def emit_w_chunk(oc):
                ws = wstage.tile([P, H], f32, name="ws", tag="ws")
                nc.sync.dma_start(ws, W[oc * P : (oc + 1) * P, :])
                wb = wstage.tile([P, H], bf16, name="wb16", tag="wb")
                nc.scalar.mul(wb, ws, W_SCALE)
                pt = psum.tile([P, HC, P], bf16, name="pt16", tag="pclo", bufs=2)
                for hc in range(HC):
                    nc.tensor.transpose(
                        pt[:, hc, :], wb[:, hc * P : (hc + 1) * P], ident
                    )
                nc.any.tensor_copy(Wt[:, :, oc * P : (oc + 1) * P], pt)

            def emit_x_chunk(sc):
                xs = stage.tile([P, H], f32, name="xs", tag="xs")
                nc.sync.dma_start(xs, x[sc * P : (sc + 1) * P, :])
                nc.scalar.copy(x_h8[:, sc, :], xs)
                lo16 = stage.tile([P, H], bf16, name="lo16", tag="lo16")
                sub_eng.tensor_sub(lo16, xs, x_h8[:, sc, :])
                nc.scalar.mul(x_l8[:, sc, :], lo16, 256.0)
                if XTR_F8:
                    # fp8 PE transpose requires output element step of 2
                    pt = psum.tile([P, HC, P, 2], f8, name="pt8", tag="pclo", bufs=2)
                    for hc in range(HC):
                        nc.tensor.transpose(
                            pt[:, hc, :, 0],
                            x_h8[:, sc, hc * P : (hc + 1) * P],
                            ident_f8,
                        )
                    nc.any.tensor_copy(
                        xT[:, :, sc * P : (sc + 1) * P], pt[:, :, :, 0]
                    )
                else:
                    xh16 = stage.tile([P, H], bf16, name="xh16", tag="xh16")
                    nc.gpsimd.tensor_copy(xh16, xs)
                    pt = psum.tile([P, HC, P], bf16, name="pt16", tag="pclo", bufs=2)
                    for hc in range(HC):
                        nc.tensor.transpose(
                            pt[:, hc, :], xh16[:, hc * P : (hc + 1) * P], ident
                        )
                    nc.any.tensor_copy(xT[:, :, sc * P : (sc + 1) * P], pt)

            emitted_w = 0
            for sc in range(SQ):
                emit_x_chunk(sc)
                while emitted_w < min(HC, sc + 1):
                    emit_w_chunk(emitted_w)
                    emitted_w += 1
            while emitted_w < HC:
                emit_w_chunk(emitted_w)
                emitted_w += 1

            # --- Phase L: outT[o, s] = (Wt.T @ xT)/32 + b ------------------
            # two ns-groups: group g covers s columns [g*S/2, (g+1)*S/2) and
            # only needs the first/second half of the x chunks, so group 0's
            # matmuls start while the second half of x is still loading.
            for g in range(2):
                nsg = range(g * NK // 2, (g + 1) * NK // 2)
                for oc in range(HC):
                    pls = {
                        ns: psum.tile(
                            [P, F], f32, name=f"pl{ns}", tag="mm", bufs=4
                        )
                        for ns in nsg
                    }
                    for i in range(HC // 2):
                        for ns in nsg:
                            nc.tensor.matmul(
                                pls[ns],
                                lhsT=Wt[
                                    :, 2 * i : 2 * i + 2, oc * P : (oc + 1) * P
                                ],
                                rhs=xT[:, 2 * i : 2 * i + 2, ns * F : (ns + 1) * F],
                                start=(i == 0),
                                stop=(i == HC // 2 - 1),
                                perf_mode=mybir.MatmulPerfMode.DoubleRow,
                            )
                    for ns in nsg:
                        # outT = psum / W_SCALE + b
                        nc.vector.tensor_scalar(
                            outT[:, oc, ns * F : (ns + 1) * F],
                            pls[ns],
                            1.0 / W_SCALE,
                            b_sb[:, oc : oc + 1],
                            op0=mybir.AluOpType.mult,
                            op1=mybir.AluOpType.add,
                        )

        # --- Phase A: per q-block score/softmax/context --------------------
        # The LAST block's softmax chain is hoisted to right after the linear,
        # with only its context at the end — the pipeline tail then drains
        # into ready context matmuls instead of waiting on a softmax chain.
        with tc.tile_pool(name=f"attn{it}", bufs=1) as ap:

            def emit_ss(qb, sfx="", nbufs=3):
                mx = ap.tile([P, NK], f32, name="mx" + sfx, tag="mx" + sfx, bufs=nbufs)
                pss = [
                    psum.tile([P, F], f32, name=f"ps{nk}", tag="mm", bufs=4)
                    for nk in range(NK)
                ]
                for i in range(HC // 2):
                    for nk in range(NK):
                        nc.tensor.matmul(
                            pss[nk],
                            lhsT=outT[:, 2 * i : 2 * i + 2, qb * P : (qb + 1) * P],
                            rhs=outT[:, 2 * i : 2 * i + 2, nk * F : (nk + 1) * F],
                            start=(i == 0),
                            stop=(i == HC // 2 - 1),
                            perf_mode=mybir.MatmulPerfMode.DoubleRow,
                        )
                attn_sb = ap.tile(
                    [P, S], bf16, name="attn_sb" + sfx, tag="attn" + sfx, bufs=nbufs
                )
                ssum2 = ap.tile(
                    [P, NK], f32, name="ssum2" + sfx, tag="ssum" + sfx, bufs=nbufs
                )
                nmx = ap.tile([P, 1], f32, name="nmx" + sfx, tag="nmx" + sfx, bufs=nbufs)
                if EXP_PSUM:
                    for nk in range(NK):
                        nc.vector.reduce_max(
                            mx[:, nk : nk + 1], pss[nk], axis=mybir.AxisListType.X
                        )
                    nc.vector.reduce_max(
                        nmx, mx, axis=mybir.AxisListType.X, negate=True
                    )
                    for nk in range(NK):
                        nc.scalar.activation(
                            attn_sb[:, nk * F : (nk + 1) * F],
                            pss[nk],
                            mybir.ActivationFunctionType.Exp,
                            bias=nmx,
                            scale=1.0,
                            accum_out=ssum2[:, nk : nk + 1],
                        )
                else:
                    sc_f32 = ap.tile(
                        [P, S], f32, name="sc_f32" + sfx, tag="sc" + sfx, bufs=nbufs
                    )
                    for nk in range(NK):
                        nc.any.tensor_copy(sc_f32[:, nk * F : (nk + 1) * F], pss[nk])
                        nc.vector.reduce_max(
                            mx[:, nk : nk + 1], pss[nk], axis=mybir.AxisListType.X
                        )
                    nc.vector.reduce_max(
                        nmx, mx, axis=mybir.AxisListType.X, negate=True
                    )
                    for h2 in range(4):
                        nc.scalar.activation(
                            attn_sb[:, h2 * (S // 4) : (h2 + 1) * (S // 4)],
                            sc_f32[:, h2 * (S // 4) : (h2 + 1) * (S // 4)],
                            mybir.ActivationFunctionType.Exp,
                            bias=nmx,
                            scale=1.0,
                            accum_out=ssum2[:, h2 : h2 + 1],
                        )
                rsum = ap.tile(
                    [P, 1], f32, name="rsum" + sfx, tag="rsum" + sfx, bufs=nbufs
                )
                rsum256 = ap.tile(
                    [P, 1], f32, name="rsum256" + sfx, tag="rs2" + sfx, bufs=nbufs
                )
                ssum = ap.tile(
                    [P, 1], f32, name="ssum" + sfx, tag="ssum1" + sfx, bufs=nbufs
                )
                nc.vector.reduce_sum(ssum, ssum2, axis=mybir.AxisListType.X)
                nc.vector.reciprocal(rsum, ssum)
                nc.vector.tensor_scalar_mul(rsum256, rsum, 1.0 / 256.0)
                attnT = ap.tile(
                    [P, SQ, P], bf16, name="attnT" + sfx, tag="attnT" + sfx, bufs=nbufs
                )
                attnT_f8 = ap.tile(
                    [P, SQ, P], f8, name="attnT_f8" + sfx, tag="aT8" + sfx, bufs=nbufs
                )
                for h2 in range(2):
                    half = slice(h2 * (SQ // 2), (h2 + 1) * (SQ // 2))
                    nc.sync.dma_start_transpose(
                        attnT[:, half, :],
                        attn_sb[:, h2 * (S // 2) : (h2 + 1) * (S // 2)],
                    )
                    at8_eng.tensor_copy(attnT_f8[:, half, :], attnT[:, half, :])
                return attnT_f8, rsum, rsum256

            def emit_ctx(qb, attnT_f8, rsum, rsum256, sfx="", nbufs=3,
                         hi_tag="pctx", hi_bufs=None, lo_tag="pclo", lo_bufs=2):
                if hi_bufs is None:
                    hi_bufs = PCTX_BUFS
                ctx_sb = ap.tile(
                    [P, H], f32, name="ctx_sb" + sfx, tag="ctx" + sfx, bufs=nbufs
                )
                ctx_hi = ap.tile(
                    [P, H], f32, name="ctx_hi" + sfx, tag="ctxh" + sfx, bufs=nbufs
                )
                phs = [
                    psum.tile([P, F], f32, name=f"pch{hn}", tag=hi_tag, bufs=hi_bufs)
                    for hn in range(NH)
                ]
                pls = [
                    psum.tile([P, F], f32, name=f"pcl{hn}", tag=lo_tag, bufs=lo_bufs)
                    for hn in range(NH)
                ]
                for i in range(SQ // 2):
                    for hn in range(NH):
                        nc.tensor.matmul(
                            phs[hn],
                            lhsT=attnT_f8[:, 2 * i : 2 * i + 2, :],
                            rhs=x_h8[:, 2 * i : 2 * i + 2, hn * F : (hn + 1) * F],
                            start=(i == 0),
                            stop=(i == SQ // 2 - 1),
                            perf_mode=mybir.MatmulPerfMode.DoubleRow,
                        )
                        nc.tensor.matmul(
                            pls[hn],
                            lhsT=attnT_f8[:, 2 * i : 2 * i + 2, :],
                            rhs=x_l8[:, 2 * i : 2 * i + 2, hn * F : (hn + 1) * F],
                            start=(i == 0),
                            stop=(i == SQ // 2 - 1),
                            perf_mode=mybir.MatmulPerfMode.DoubleRow,
                        )
                for hn in range(NH):
                    sl = slice(hn * F, (hn + 1) * F)
                    nc.scalar.activation(
                        ctx_hi[:, sl],
                        phs[hn],
                        mybir.ActivationFunctionType.Copy,
                        scale=rsum,
                    )
                    # ctx = lo_psum * rsum/256 + ctx_hi  (one DVE op)
                    nc.vector.scalar_tensor_tensor(
                        ctx_sb[:, sl],
                        pls[hn],
                        rsum256,
                        ctx_hi[:, sl],
                        op0=mybir.AluOpType.mult,
                        op1=mybir.AluOpType.add,
                    )
                nc.sync.dma_start(out[qb * P : (qb + 1) * P, :], ctx_sb)

            last = SQ - 1
            lt = emit_ss(last, sfx="L", nbufs=1)
            for qb in range(SQ - 1):
                t = emit_ss(qb)
                emit_ctx(qb, *t)
            # last block's context on the (now idle) score psum banks so it
            # can run during block 14's softmax instead of after it
            emit_ctx(last, *lt, sfx="L", nbufs=1,
                     hi_tag="mm", hi_bufs=4, lo_tag="mm", lo_bufs=4)


def build(n_iters=1):
    """Build the per-core Bass program. Returns compiled nc."""
    nc = bacc.Bacc("TRN2", target_bir_lowering=False, debug=False, num_devices=8)
    x = nc.dram_tensor("x", [S, H], f32, kind="ExternalInput").ap()
    W = nc.dram_tensor("W", [H, H], f32, kind="ExternalInput").ap()
    b = nc.dram_tensor("b", [H], f32, kind="ExternalInput").ap()
    out = nc.dram_tensor("ctx_out", [S, H], f32, kind="ExternalOutput").ap()

    with tile.TileContext(nc) as tc:
        with ExitStack() as top:
            const = top.enter_context(tc.tile_pool(name="const", bufs=1))
            ident = const.tile([P, P], bf16, name="ident")
            make_identity(nc, ident)
            ident_f8 = const.tile([P, P], f8, name="ident_f8")
            make_identity(nc, ident_f8)
            b_sb = const.tile([P, HC], f32, name="b_sb")
            nc.sync.dma_start(b_sb, b.rearrange("(c p) -> p c", p=P))
            psum = top.enter_context(
                tc.tile_pool(name="psum", bufs=1, space="PSUM")
            )
            for it in range(n_iters):
                emit_iteration(
                    nc, tc, x, W, b, out, psum, const, ident, ident_f8, b_sb, it
                )

    nc.compile()
    return nc


_CACHED = {}


def _get_nc(n_iters=1):
    if n_iters not in _CACHED:
        _CACHED[n_iters] = build(n_iters)
    return _CACHED[n_iters]


def kernel(lstm_out: np.ndarray, W: np.ndarray, b: np.ndarray) -> np.ndarray:
    """Full-input entry point: shards batch over 8 cores, returns [B,S,H] f32."""
    nc = _get_nc()
    lstm_out = np.ascontiguousarray(lstm_out, dtype=np.float32)
    Wc = np.ascontiguousarray(W, dtype=np.float32)
    bc = np.ascontiguousarray(b, dtype=np.float32)
    in_maps = [{"x": lstm_out[c], "W": Wc, "b": bc} for c in range(B)]
    res = run_bass_kernel_spmd(nc, in_maps, core_ids=list(range(B)))
    return np.stack([res.results[c]["ctx_out"] for c in range(B)], axis=0)


if __name__ == "__main__":
    rng = np.random.default_rng(0)
    xs = rng.standard_normal((B, S, H), dtype=np.float32)
    Ws = (rng.standard_normal((H, H), dtype=np.float32) / np.sqrt(H)).astype(
        np.float32
    )
    bs = (0.01 * rng.standard_normal(H)).astype(np.float32)
    r = kernel(xs, Ws, bs)
    print(r.shape, r.dtype)
